# revision 2
# baseline (speedup 1.0000x reference)
"""Trainium2 Bass kernel for the EventScopeGuardHead pair-MLP edge scorer.

Same math/decomposition as the baseline kernel.py, rebalanced across engines:
  - the |h_i - h_j| feature is rewritten via |a-b| = a + b - 2*min(a,b):
    the w1c.T h_i part rides the e-matmul alpha row, the w1c.T h_j part is
    folded into the d-matmul weights (w1b+w1c), and only min(h_i,h_j) is
    computed per pair -- one single-op DVE tensor_scalar(min) per i in bf16
    4x mode (~127ns) instead of two ACT Abs-with-bias calls (~800ns/blk).
  - All layer-1 matmul operands go bf16: stationaries get the FWL fast path,
    moving side streams 1 col/cycle at any FD (f32r needs FD>=256).
  - e-matmul (edge-logit features) goes from one K=7 f32r matmul (512 PE
    columns/blk) to four concurrent K=5 bf16 row-tiled matmuls
    (tile_position=(32q,0)), each streaming 128 cols through its own
    row-group: ~4x less PE time for that term. Stationary per chunk =
    [w1e(4 rows); alpha_i + b1], alpha rows DMA-updated per block into a
    4-slot rotating stationary table.
  - ACT only does the relu (one FD=512 PSUM->SBUF op per block).
"""

import sys

import numpy as np

sys.path.insert(0, "/opt/trn_rl_repo")

B, N, D, HID = 4, 256, 128, 128
NCORES = 8
NBLK = N // 4        # 64 blocks of (2 i's x 256 j) per core
BLK_PER_GRP = 8      # e-feature staging granularity: 16 i's
NGRP = NBLK // BLK_PER_GRP
NSLOT = 4            # rotating e-stationary tables


def _build_nc(reps=0, unroll=1, ablate=()):
    import contextlib
    import concourse.bass as bass
    import concourse.tile as tile
    from concourse import bacc, mybir

    f32 = mybir.dt.float32
    bf16 = mybir.dt.bfloat16
    f16 = mybir.dt.float16
    AF = mybir.ActivationFunctionType
    ALU = mybir.AluOpType

    nc = bacc.Bacc("TRN2", target_bir_lowering=False, debug=False)

    hTb_d = nc.dram_tensor("hTb", [D, N], bf16, kind="ExternalInput").ap()
    hTi_d = nc.dram_tensor("hTi", [D, 128], f32, kind="ExternalInput").ap()
    br_d = nc.dram_tensor("base_r", [128, N], bf16, kind="ExternalInput").ap()
    cr_d = nc.dram_tensor("comp_r", [128, N], bf16, kind="ExternalInput").ap()
    w1c_d = nc.dram_tensor("w1c", [D, HID], bf16, kind="ExternalInput").ap()
    w1d_d = nc.dram_tensor("w1d", [D, HID], bf16, kind="ExternalInput").ap()
    w1b_d = nc.dram_tensor("w1b", [D, HID], bf16, kind="ExternalInput").ap()
    w1e_d = nc.dram_tensor("w1e", [4, HID], bf16, kind="ExternalInput").ap()
    ab_d = nc.dram_tensor("ab", [128, HID], bf16, kind="ExternalInput").ap()
    w2_d = nc.dram_tensor("w2", [HID, 1], f16, kind="ExternalInput").ap()
    out_d = nc.dram_tensor("out_raw", [128, 256], f32, kind="ExternalOutput").ap()

    with tile.TileContext(nc) as tc:
        with (
            tc.tile_pool(name="const", bufs=1) as cpool,
            tc.tile_pool(name="cfeat", bufs=4) as cf_pool,
            tc.tile_pool(name="ldp", bufs=8) as ld_pool,
            tc.tile_pool(name="hdn", bufs=5) as hdn_pool,
            tc.tile_pool(name="efeat", bufs=2) as ef_pool,
            tc.tile_pool(name="pre", bufs=6, space="PSUM") as pre_pool,
            tc.tile_pool(name="l2p", bufs=1, space="PSUM") as l2_pool,
        ):
            # ---- load constants/persistent tiles ----
            hTb = cpool.tile([D, N], bf16)
            nc.sync.dma_start(hTb[:], hTb_d)
            hTi = cpool.tile([D, 128], f32)
            nc.sync.dma_start(hTi[:], hTi_d)
            brb = cpool.tile([128, N], bf16)
            nc.sync.dma_start(brb[:], br_d)
            crb = cpool.tile([128, N], bf16)
            nc.sync.dma_start(crb[:], cr_d)
            w1cb = cpool.tile([D, HID], bf16)
            nc.sync.dma_start(w1cb[:], w1c_d)
            w1db = cpool.tile([D, HID], bf16)
            nc.sync.dma_start(w1db[:], w1d_d)
            w1bb = cpool.tile([D, HID], bf16)
            nc.sync.dma_start(w1bb[:], w1b_d)
            w1eb = cpool.tile([4, HID], bf16)
            nc.sync.dma_start(w1eb[:], w1e_d)
            aTb = cpool.tile([128, HID], bf16)
            nc.sync.dma_start(aTb[:], ab_d)
            w2 = cpool.tile([HID, 1], f16)
            nc.sync.dma_start(w2[:], w2_d)

            # ---- prologue compute: sigmoids of the pre-clipped logits ----
            sgbb = cpool.tile([128, N], bf16)
            nc.scalar.activation(sgbb[:], brb[:], AF.Sigmoid)
            sgcb = cpool.tile([128, N], bf16)
            nc.scalar.activation(sgcb[:], crb[:], AF.Sigmoid)

            # e-feature staging helper: one group = 8 blocks, one flat DMA
            # per feature. br/cr rows are in original local-i order, whose
            # row-major flatten (16 rows x 256 j) exactly equals the
            # e-moving column order (g, ii, j). Indicator rows 4/5 and the
            # zero padding are persistent -- staging touches rows 0..3 only.
            def stage_ef(gi, tile):
                for k, srcf in enumerate((brb, sgbb, crb, sgcb)):
                    nc.sync.dma_start(
                        tile[k:k + 1, :],
                        srcf[16 * gi:16 * gi + 16, :],
                    )

            # indicator rows: row 0 selects the i0 half (cols 0:256), row 1
            # the i1 half -- they gate the two alpha rows of the stationary.
            ind2 = cpool.tile([2, 512], bf16)
            nc.vector.memset(ind2[:], 0.0)
            nc.vector.memset(ind2[0:1, 0:256], 1.0)
            nc.sync.dma_start(ind2[1:2, 256:512], ind2[0:1, 0:256])

            # e-matmul stationary table for all 64 blocks, built once and
            # zero-padded to K=128 so the e-matmul is a standard full-K
            # matmul (tiny-K matmuls run far below rate on this runtime).
            # Block blk uses es_all[:, blk*128:(blk+1)*128]: rows 0..3 =
            # w1e, rows 4/5 = alpha'_{i0/i1} = (w1a+w1c).T h_i + b1.
            es_all = cpool.tile([128, NBLK * HID], bf16)
            nc.vector.memset(es_all[:], 0.0)
            nc.sync.dma_start(
                es_all[0:4, :].rearrange("r (blk c) -> r blk c", c=HID),
                w1eb[:, None, :].broadcast_to((4, NBLK, HID)),
            )
            nc.sync.dma_start(es_all[4:5, :], aTb[0:64, :])
            nc.sync.dma_start(es_all[5:6, :], aTb[64:128, :])

            # two persistent ping-pong e-feature tiles, zero-padded once;
            # group staging only rewrites rows 0..5.
            ef_a = cpool.tile([128, BLK_PER_GRP * 512], bf16)
            ef_b = cpool.tile([128, BLK_PER_GRP * 512], bf16)
            nc.vector.memset(ef_a[:], 0.0)
            nc.vector.memset(ef_b[:], 0.0)
            for eft in (ef_a, ef_b):
                nc.sync.dma_start(
                    eft[4:6, :].rearrange("p (g c) -> p g c", c=512),
                    ind2[:, None, :].broadcast_to((2, BLK_PER_GRP, 512)),
                )
            stage_ef(0, ef_a)

            # ---- main loop ----
            l2ps = l2_pool.tile([128, 256], f32)
            L2_DELAY = 2

            def emit_l2(lblk, lhdn):
                if "l2" in ablate and lblk != 0:
                    return
                for c in range(4):
                    nc.tensor.matmul(
                        l2ps[:, 4 * lblk + c:4 * lblk + c + 1],
                        lhdn[:, c * 128:(c + 1) * 128],
                        w2[:],
                        start=True, stop=True,
                    )

            rep_ctx = tc.For_i(0, reps, 1) if reps else contextlib.nullcontext()
            with rep_ctx:
             for _u in range(unroll):
              l2q = []
              next_ef = ef_a
              for blk in range(NBLK):
                  i0, i1 = blk, 64 + blk  # hTi cols are even/odd-grouped

                  if blk % BLK_PER_GRP == 0:
                      # rotate in the prefetched group, prefetch the next
                      if "estage" in ablate:
                          ef = ef_a
                      else:
                          ef = next_ef
                          ngi = (blk // BLK_PER_GRP + 1) % NGRP
                          next_ef = ef_a if ngi % 2 == 0 else ef_b
                          stage_ef(ngi, next_ef)

                  # min(h_i, h_j): the only per-pair piece of |h_i - h_j|
                  cfeat = cf_pool.tile([D, 512], bf16, tag="cf")
                  if "cfeat" not in ablate:
                      nc.vector.tensor_scalar(
                          cfeat[:, 0:256], hTb[:], hTi[:, i0:i0 + 1], None,
                          ALU.min,
                      )
                      nc.vector.tensor_scalar(
                          cfeat[:, 256:512], hTb[:], hTi[:, i1:i1 + 1], None,
                          ALU.min,
                      )
                  else:
                      nc.vector.tensor_scalar(
                          cfeat[:, 0:1], hTb[:, 0:1], hTi[:, i0:i0 + 1], None,
                          ALU.min,
                      )

                  # d-matmul weights: w1d * h_i + w1b  (bf16 out -> FWL)
                  ld0 = ld_pool.tile([D, HID], bf16, tag="ld")
                  ld1 = ld_pool.tile([D, HID], bf16, tag="ld")
                  if "stt" not in ablate:
                      nc.vector.scalar_tensor_tensor(
                          ld0[:], w1db[:], hTi[:, i0:i0 + 1], w1bb[:],
                          ALU.mult, ALU.add,
                      )
                      nc.vector.scalar_tensor_tensor(
                          ld1[:], w1db[:], hTi[:, i1:i1 + 1], w1bb[:],
                          ALU.mult, ALU.add,
                      )
                  else:
                      nc.vector.scalar_tensor_tensor(
                          ld0[:, 0:1], w1db[:, 0:1], hTi[:, i0:i0 + 1],
                          w1bb[:, 0:1], ALU.mult, ALU.add,
                      )
                      nc.vector.scalar_tensor_tensor(
                          ld1[:, 0:1], w1db[:, 0:1], hTi[:, i1:i1 + 1],
                          w1bb[:, 0:1], ALU.mult, ALU.add,
                      )

                  pre = pre_pool.tile([128, 512], f32)
                  g = blk % BLK_PER_GRP
                  mms = []
                  if "edup" in ablate:
                      mms.append((pre[:], w1cb[:], cfeat[:]))
                  elif "emm" not in ablate:
                      mms.append((pre[:],
                                  es_all[:, blk * HID:(blk + 1) * HID],
                                  ef[:, g * 512:(g + 1) * 512]))
                  if "dmm" not in ablate:
                      mms.append((pre[:, 0:256], ld0[:], hTb[:]))
                      mms.append((pre[:, 256:512], ld1[:], hTb[:]))
                  if "cmm" not in ablate:
                      mms.append((pre[:], w1cb[:], cfeat[:]))
                  if not mms:
                      mms.append((pre[:], w1cb[:], cfeat[:]))
                  for mi, (o, l, r) in enumerate(mms):
                      nc.tensor.matmul(
                          o, l, r, start=(mi == 0), stop=(mi == len(mms) - 1),
                          skip_group_check=True,
                      )

                  # relu: PSUM -> SBUF on ACT; fp16 for layer-2 FWL
                  hdn = hdn_pool.tile([128, 512], f16)
                  if "relu" not in ablate:
                      nc.scalar.activation(hdn[:], pre[:], AF.Relu)
                  else:
                      nc.scalar.activation(hdn[:, 0:1], pre[:, 0:1], AF.Relu)

                  # layer 2, delayed 2 blocks so the PE never waits on relu
                  l2q.append((blk, hdn))
                  if len(l2q) > L2_DELAY:
                      emit_l2(*l2q.pop(0))

              for item in l2q:
                  emit_l2(*item)

            out_sb = cpool.tile([128, 256], f32)
            nc.vector.tensor_copy(out_sb[:], l2ps[:])
            nc.sync.dma_start(out_d, out_sb[:])

    nc.compile()
    return nc


_NC_CACHE = {}


def _get_nc():
    if "nc" not in _NC_CACHE:
        _NC_CACHE["nc"] = _build_nc()
    return _NC_CACHE["nc"]


def _bf16():
    from concourse import mybir
    return mybir.dt.np(mybir.dt.bfloat16)


def make_in_maps(node_latents, base_edge_logits, completion_logits, W1, b1, W2, b2):
    bf = _bf16()
    base_c = np.clip(base_edge_logits, -20.0, 20.0)
    comp_c = np.clip(completion_logits, -20.0, 20.0)
    # |a-b| = a + b - 2*min(a,b): the "a" (h_i) part goes into the alpha
    # row, the "b" (h_j) part into the d-matmul weights, min is per-pair.
    ab_full = node_latents @ (W1[0:128] + W1[256:384]) + b1[None, None, :]
    in_maps = []
    for core in range(NCORES):
        b, ihalf = core // 2, core % 2
        hT = np.ascontiguousarray(node_latents[b].T).astype(np.float32)
        hTi_n = hT[:, ihalf * 128:(ihalf + 1) * 128]
        hTi = np.ascontiguousarray(
            np.concatenate([hTi_n[:, 0::2], hTi_n[:, 1::2]], axis=1)
        )
        abn = ab_full[b, ihalf * 128:(ihalf + 1) * 128]
        ab = np.concatenate([abn[0::2], abn[1::2]], axis=0)
        in_maps.append({
            "hTb": hT.astype(bf),
            "hTi": hTi,
            "base_r": np.ascontiguousarray(
                base_c[b, ihalf * 128:(ihalf + 1) * 128, :]
            ).astype(bf),
            "comp_r": np.ascontiguousarray(
                comp_c[b, ihalf * 128:(ihalf + 1) * 128, :]
            ).astype(bf),
            "w1c": np.ascontiguousarray(-2.0 * W1[256:384]).astype(bf),
            "w1d": np.ascontiguousarray(W1[384:512]).astype(bf),
            "w1b": np.ascontiguousarray(W1[128:256] + W1[256:384]).astype(bf),
            "w1e": np.ascontiguousarray(W1[512:516]).astype(bf),
            "ab": np.ascontiguousarray(ab).astype(bf),
            "w2": np.ascontiguousarray(0.5 * W2).astype(np.float16),
        })
    return in_maps


def decode_core_out(arr):
    """[128 p, 256 (blk,ii,jh)] psum layout -> [128 i_local, 256 j]."""
    return np.ascontiguousarray(
        arr.reshape(128, NBLK, 2, 2).transpose(1, 2, 3, 0).reshape(128, 256)
    )


def assemble(results, b2):
    out = np.zeros((B, N, N), np.float32)
    for core in range(NCORES):
        b, ihalf = core // 2, core % 2
        out[b, ihalf * 128:(ihalf + 1) * 128, :] = decode_core_out(
            results[core]["out_raw"]
        )
    out = out + out.transpose(0, 2, 1) + np.float32(b2.reshape(-1)[0])
    idx = np.arange(N)
    out[:, idx, idx] = np.float32(-1e9)
    return out.astype(np.float32)


def _get_runner():
    """Build (once) a reusable jitted SPMD callable over 8 cores."""
    if "runner" in _NC_CACHE:
        return _NC_CACHE["runner"]

    import jax
    import numpy as _np
    from jax.sharding import Mesh, PartitionSpec
    from jax.experimental.shard_map import shard_map
    from concourse import mybir
    from concourse.bass2jax import (
        _bass_exec_p, install_neuronx_cc_hook, partition_id_tensor,
    )

    nc = _get_nc()
    install_neuronx_cc_hook()

    pid_name = nc.partition_id_tensor.name if nc.partition_id_tensor else None
    in_names, out_names, out_avals, zero_outs = [], [], [], []
    for alloc in nc.m.functions[0].allocations:
        if not isinstance(alloc, mybir.MemoryLocationSet):
            continue
        name = alloc.memorylocations[0].name
        if alloc.kind == "ExternalInput":
            if name == pid_name:
                continue
            in_names.append(name)
        elif alloc.kind == "ExternalOutput":
            shape = tuple(alloc.tensor_shape)
            dtype = mybir.dt.np(alloc.dtype)
            out_names.append(name)
            out_avals.append(jax.core.ShapedArray(shape, dtype))
            zero_outs.append(_np.zeros(shape, dtype))
    n_params, n_outs = len(in_names), len(out_avals)

    def _body(*args):
        operands = list(args)
        names = in_names + out_names
        if pid_name is not None:
            operands.append(partition_id_tensor())
            names = names + [pid_name]
        outs = _bass_exec_p.bind(
            *operands,
            out_avals=tuple(out_avals),
            in_names=tuple(names),
            out_names=tuple(out_names),
            lowering_input_output_aliases=(),
            sim_require_finite=True,
            sim_require_nnan=True,
            nc=nc,
        )
        return tuple(outs)

    devices = jax.devices()[:NCORES]
    mesh = Mesh(_np.asarray(devices), ("core",))
    specs = (PartitionSpec("core"),) * (n_params + n_outs)
    sharded = jax.jit(
        shard_map(_body, mesh=mesh, in_specs=specs,
                  out_specs=(PartitionSpec("core"),) * n_outs,
                  check_rep=False),
        donate_argnums=tuple(range(n_params, n_params + n_outs)),
        keep_unused=True,
    )

    def run(in_maps):
        concat_in = [
            _np.concatenate([_np.asarray(in_maps[c][nm]) for c in range(NCORES)],
                            axis=0)
            for nm in in_names
        ]
        concat_zeros = [
            _np.zeros((NCORES * z.shape[0], *z.shape[1:]), z.dtype)
            for z in zero_outs
        ]
        out_arrs = sharded(*concat_in, *concat_zeros)
        return [
            {nm: _np.asarray(out_arrs[i]).reshape(NCORES, *out_avals[i].shape)[c]
             for i, nm in enumerate(out_names)}
            for c in range(NCORES)
        ]

    _NC_CACHE["runner"] = run
    return run


def kernel(**inputs):
    run = _get_runner()
    in_maps = make_in_maps(
        np.asarray(inputs["node_latents"]),
        np.asarray(inputs["base_edge_logits"]),
        np.asarray(inputs["completion_logits"]),
        np.asarray(inputs["W1"]),
        np.asarray(inputs["b1"]),
        np.asarray(inputs["W2"]),
        np.asarray(inputs["b2"]),
    )
    results = run(in_maps)
    return assemble(results, np.asarray(inputs["b2"]))

